# revision 5
# baseline (speedup 1.0000x reference)
"""Gabor-modulated conv-weight synthesis on 8 Trainium2 NeuronCores.

Computes out[g*CO + co, ci, h, w] = gabor(theta[g], lam[g])[h, w] * x[co, ci, h, w]
for x: [512, 512, 9, 9] f32, theta/lam: [4] f32  ->  out: [2048, 512, 9, 9] f32.

Sharding: x along C_out into 8 shards of 64; theta/lam replicated; each core
produces its [4, 64, 512, 9, 9] output slice with no communication.

The problem is pure DMA-bound (per core: read the x shard, write 4 scaled
copies).  Design notes, in the order they bought time:

- fp16 end-to-end (tolerance is 2e-2; fp16 rounding contributes ~1e-3):
  host converts x to fp16, device streams fp16, host upcasts the result.
  Halves HBM traffic to 5.3 MB in + 21.2 MB out per core.
- The [4, 81] Gabor table is synthesized on the host (332 flops from 8
  input scalars) so the device program has no serial synthesis prologue.
- Everything rides the two HWDGE rings (SP/ACT); SWDGE descriptor
  generation on the gpsimd Q7 is far too slow and also drags down all 16
  SDMA engines.  Loads are interleaved into the store FIFOs so both rings
  stay saturated (~420 GB/s aggregate mid-kernel).
- SDMA engine #15 moves bytes ~17% slower than engines 0-14 (known
  erratum), and with a uniform layout it tail-drains alone for ~14 us.
  Its partitions ({92-95, 124-127} by the port swizzle) therefore get 211
  rows while the other 120 partitions get 259 (120*259 + 8*211 = 32768).
  The device DRAM layout is [common: 128 partitions x 211 rows] ++
  [extra: 120 fast partitions x 48 rows]; the host permutes x into this
  layout and un-permutes the output, so all device APs stay affine.
- DVE multiplies run in fp16 2x perf mode (packed 2-byte last dim)
  against a step-0-broadcast view of the Gabor row; small first chunk for
  an early first store; the fast-only extra chunk is last, shortening the
  slow engine's tail further.
"""

import numpy as np

import concourse.bass as bass
import concourse.bacc as bacc
import concourse.mybir as mybir
from concourse.tile import TileContext
from concourse.bass_utils import run_bass_kernel_spmd

N_CORES = 8
G = 4
CO, CI, H, W = 512, 512, 9, 9
HW = H * W                # 81
CO_SH = CO // N_CORES     # 64 C_out rows per core
ROWS = CO_SH * CI         # 32768 (co_local, ci) rows per core
P = 128                   # SBUF partitions
SIGMA = float(np.pi)      # Gaussian envelope std of the Gabor synthesis

# Row budget per partition: SDMA engine 15 (partitions 92-95 and 124-127)
# is ~17% slower than the rest, so its partitions carry fewer rows.
T = 211                   # rows per SLOW partition (= common block rows)
E = 48                    # extra rows per FAST partition (total 211+48=259)
NF = 120                  # number of fast partitions
assert P * T + NF * E == ROWS
CHUNKS_T = (32, 64, 64, 51)     # common-block chunking (sums to T)
NSUB_MAX = max(CHUNKS_T)
# fast partition index f -> physical partition: f for f<92, f+4 for f>=92

F16 = mybir.dt.float16
ALU = mybir.AluOpType


def build_bass():
    assert sum(CHUNKS_T) == T

    nc = bacc.Bacc("TRN2", target_bir_lowering=False, debug=False)
    x = nc.declare_dram_parameter("x", [ROWS, HW], F16, isOutput=False)
    gb = nc.declare_dram_parameter("gb", [G * HW], F16, isOutput=False)
    out = nc.declare_dram_parameter("out", [G, ROWS, HW], F16, isOutput=True)

    # common block: [128, T, 81]; extra block: [120, E, 81] (fast partitions)
    xc = x.ap()[0:P * T, :].rearrange("(p n) m -> p n m", p=P)
    xe = x.ap()[P * T:ROWS, :].rearrange("(f n) m -> f n m", f=NF)
    oc = out.ap()[:, 0:P * T, :].rearrange(
        "g (p n) m -> g p n m", p=P).transpose([1, 0, 2, 3])
    oe = out.ap()[:, P * T:ROWS, :].rearrange(
        "g (f n) m -> g f n m", f=NF).transpose([1, 0, 2, 3])

    with TileContext(nc) as tc:
        with tc.tile_pool(name="consts", bufs=1) as cpool, \
             tc.tile_pool(name="xs", bufs=len(CHUNKS_T) + 1) as xpool, \
             tc.tile_pool(name="outs", bufs=10) as opool:
            # Ring plan (FIFO order per ring, ~13.2 MB each):
            #   sync  : x0 x2 s(0,0) s(0,2) xe0 s(1,0) s(1,2) ...
            #   scalar: gb x1 x3 s(0,1) s(0,3) xe1 s(1,1) s(1,3) ...
            gbt = cpool.tile([P, G * HW], F16)
            nc.scalar.dma_start(gbt, gb.ap().unsqueeze(0).broadcast_to([P, G * HW]))

            xtiles = []
            chunk_off = []
            n0 = 0
            for i, ns in enumerate(CHUNKS_T):
                xtiles.append(xpool.tile([P, NSUB_MAX * HW], F16, tag="x",
                                         name=f"xt{i}"))
                chunk_off.append(n0)
                n0 += ns

            def loadc(i):
                ns = CHUNKS_T[i]
                eng = nc.sync if i % 2 == 0 else nc.scalar
                eng.dma_start(
                    xtiles[i][:, 0:ns * HW].rearrange("p (n m) -> p n m", m=HW),
                    xc[:, chunk_off[i]:chunk_off[i] + ns, :],
                )

            loadc(0)
            loadc(1)
            loadc(2)
            loadc(3)

            # extra (fast-only) chunk: loads fill partitions 0-91 and
            # 96-123; the slow partitions are memset so the full-width
            # multiply reads initialized SBUF (their lanes are never stored)
            xte = xpool.tile([P, E * HW], F16, tag="x", name="xte")
            xtev = xte.rearrange("p (n m) -> p n m", m=HW)

            def gb_bc(g, ns, p0=0, p1=P):  # [.., 81] -> [.., ns, 81] step-0
                return gbt[p0:p1, g * HW:(g + 1) * HW].unsqueeze(1).broadcast_to(
                    [p1 - p0, ns, HW]
                )

            # ---- streaming broadcast-multiply, stores alternate rings ----
            s = 0

            def ring():
                nonlocal s
                eng = nc.sync if s % 2 == 0 else nc.scalar
                s += 1
                return eng

            for i, ns in enumerate(CHUNKS_T):
                n0 = chunk_off[i]
                xtv = xtiles[i][:, 0:ns * HW].rearrange("p (n m) -> p n m", m=HW)
                for g in range(G):
                    ot = opool.tile([P, NSUB_MAX * HW], F16, tag="o")
                    otv = ot[:, 0:ns * HW].rearrange("p (n m) -> p n m", m=HW)
                    nc.vector.tensor_tensor(otv, xtv, gb_bc(g, ns), ALU.mult)
                    ring().dma_start(oc[:, g, n0:n0 + ns, :], otv)
                if i == 0:
                    # extra-chunk loads enter the ring FIFOs here; the
                    # whole tile is memset first so the slow partitions
                    # (92-95, 124-127, never loaded or stored) are
                    # initialized for the full-width multiply.  Compute
                    # ops must start on 32-partition boundaries, so the
                    # memset can't target just those strips.
                    nc.vector.memset(xte[:, :], 0.0)
                    nc.sync.dma_start(xtev[0:92], xe[0:92])
                    nc.scalar.dma_start(xtev[96:124], xe[92:120])

            for g in range(G):
                ot = opool.tile([P, NSUB_MAX * HW], F16, tag="o")
                otv = ot[:, 0:E * HW].rearrange("p (n m) -> p n m", m=HW)
                nc.vector.tensor_tensor(otv, xtev, gb_bc(g, E), ALU.mult)
                ring().dma_start(oe[0:92, g], otv[0:92])
                ring().dma_start(oe[92:120, g], otv[96:124])
    nc.finalize()
    return nc


def make_gabor(theta, lam):
    """[G, 81] f32 Gabor filters, mirroring the reference synthesis."""
    ys = np.arange(H, dtype=np.float32) - (H - 1) / 2.0
    xs = np.arange(W, dtype=np.float32) - (W - 1) / 2.0
    y, x = np.meshgrid(ys, xs, indexing="ij")
    th = theta[:, None, None].astype(np.float32)
    l = lam[:, None, None].astype(np.float32)
    xr = x[None] * np.cos(th) + y[None] * np.sin(th)
    yr = -x[None] * np.sin(th) + y[None] * np.cos(th)
    env = np.exp(-(xr ** 2 + yr ** 2) / (2.0 * np.float32(SIGMA) ** 2))
    g = env * np.cos(2.0 * np.float32(np.pi) * xr * l)
    return g.reshape(G, HW).astype(np.float32)


def _dev_order():
    """Original-row index for each device-layout row (length ROWS)."""
    fast = [p for p in range(P) if not (92 <= p < 96 or 124 <= p < 128)]
    rp = np.full(P, T + E, dtype=np.int64)
    rp[[92, 93, 94, 95, 124, 125, 126, 127]] = T
    a = np.concatenate([[0], np.cumsum(rp)[:-1]])   # first original row per p
    common = (a[:, None] + np.arange(T)[None, :]).reshape(-1)       # [128*T]
    extra = (a[fast][:, None] + T + np.arange(E)[None, :]).reshape(-1)
    return np.concatenate([common, extra])


DEV_ORDER = _dev_order()

_NC = None
TRACE = False          # set True by the local test harness for NTFF timing
LAST_RESULT = None     # BassKernelResults of the most recent run


def kernel(x, theta, lam):
    global _NC
    if _NC is None:
        _NC = build_bass()
    x = np.ascontiguousarray(np.asarray(x, dtype=np.float32))
    theta = np.asarray(theta, dtype=np.float32).reshape(G)
    lam = np.asarray(lam, dtype=np.float32).reshape(G)
    x16 = x.astype(np.float16)
    gb16 = make_gabor(theta, lam).astype(np.float16).reshape(G * HW)

    in_maps = []
    for m in range(N_CORES):
        shard = x16[m * CO_SH:(m + 1) * CO_SH].reshape(ROWS, HW)
        in_maps.append({"x": np.ascontiguousarray(shard[DEV_ORDER]), "gb": gb16})

    global LAST_RESULT
    LAST_RESULT = run_bass_kernel_spmd(
        _NC, in_maps, list(range(N_CORES)), trace=TRACE
    )
    res = LAST_RESULT.results

    out = np.empty((G, CO, CI, H, W), dtype=np.float32)
    shard_out = np.empty((G, ROWS, HW), dtype=np.float32)
    for m in range(N_CORES):
        shard_out[:, DEV_ORDER, :] = res[m]["out"]
        out[:, m * CO_SH:(m + 1) * CO_SH] = shard_out.reshape(G, CO_SH, CI, H, W)
    return out.reshape(G * CO, CI, H, W)


# revision 6
# speedup vs baseline: 1.2848x; 1.2848x over previous
"""Gabor-modulated conv-weight synthesis on 8 Trainium2 NeuronCores.

Computes out[g*CO + co, ci, h, w] = gabor(theta[g], lam[g])[h, w] * x[co, ci, h, w]
for x: [512, 512, 9, 9] f32, theta/lam: [4] f32  ->  out: [2048, 512, 9, 9] f32.

Sharding: x along C_out into 8 shards of 64; theta/lam replicated; each core
produces its [4, 64, 512, 9, 9] output slice with no communication.

The problem is pure DMA-bound (per core: read the x shard, write 4 scaled
copies).  Design notes, in the order they bought time:

- fp16 end-to-end (tolerance is 2e-2; fp16 rounding contributes ~1e-3):
  host converts x to fp16, device streams fp16, host upcasts the result.
  Halves HBM traffic to 5.3 MB in + 21.2 MB out per core.
- The [4, 81] Gabor table is synthesized on the host (332 flops from 8
  input scalars) so the device program has no serial synthesis prologue.
- Everything rides the two HWDGE rings (SP/ACT); SWDGE descriptor
  generation on the gpsimd Q7 is far too slow and also drags down all 16
  SDMA engines.  Loads are interleaved into the store FIFOs so both rings
  stay saturated (~420 GB/s aggregate mid-kernel).
- HWDGE descriptor->engine mapping (probed): a transfer's per-partition
  descriptors are split into NE equal contiguous blocks, NE = largest
  divisor of the partition count <= 16, assigned to SDMA engines 0..NE-1.
  So a 128-partition transfer gives engine e partitions [8e:8e+8].
- SDMA engine 15 moves bytes ~17% slower than engines 0-14 (known
  erratum); with a uniform layout it tail-drains alone.  Partitions
  120-127 (its block) therefore carry 211 rows vs 259 on partitions
  0-119 (120*259 + 8*211 = 32768).  The device DRAM layout is
  [common: 128 partitions x 211 rows] ++ [extra: partitions 0-119 x 48
  rows]; the extra block's 120-partition transfers split over engines
  0-14 exactly (120 = 15 blocks of 8), giving engine 15 nothing.  The
  host permutes x into this layout and un-permutes the output, so all
  device APs stay affine.
- DVE multiplies run in fp16 2x perf mode (packed 2-byte last dim)
  against a step-0-broadcast view of the Gabor row; small first chunk for
  an early first store; the fast-only extra chunk is last, shortening the
  slow engine's tail further.
"""

import numpy as np

import concourse.bass as bass
import concourse.bacc as bacc
import concourse.mybir as mybir
from concourse.tile import TileContext
from concourse.bass_utils import run_bass_kernel_spmd

N_CORES = 8
G = 4
CO, CI, H, W = 512, 512, 9, 9
HW = H * W                # 81
CO_SH = CO // N_CORES     # 64 C_out rows per core
ROWS = CO_SH * CI         # 32768 (co_local, ci) rows per core
P = 128                   # SBUF partitions
SIGMA = float(np.pi)      # Gaussian envelope std of the Gabor synthesis

NF = 120                  # fast partitions [0:120]; slow block [120:128]
T = 211                   # rows per slow partition (= common block rows)
E = 48                    # extra rows per fast partition (211 + 48 = 259)
assert P * T + NF * E == ROWS
CHUNKS_T = (32, 64, 64, 51)     # common-block chunking (sums to T)
NSUB_MAX = max(CHUNKS_T)

F16 = mybir.dt.float16
ALU = mybir.AluOpType


def build_bass():
    assert sum(CHUNKS_T) == T

    nc = bacc.Bacc("TRN2", target_bir_lowering=False, debug=False)
    x = nc.declare_dram_parameter("x", [ROWS, HW], F16, isOutput=False)
    gb = nc.declare_dram_parameter("gb", [G * HW], F16, isOutput=False)
    out = nc.declare_dram_parameter("out", [G, ROWS, HW], F16, isOutput=True)

    # common block: [128, T, 81]; extra block: [120, E, 81]
    xc = x.ap()[0:P * T, :].rearrange("(p n) m -> p n m", p=P)
    xe = x.ap()[P * T:ROWS, :].rearrange("(p n) m -> p n m", p=NF)
    oc = out.ap()[:, 0:P * T, :].rearrange(
        "g (p n) m -> g p n m", p=P).transpose([1, 0, 2, 3])
    oe = out.ap()[:, P * T:ROWS, :].rearrange(
        "g (p n) m -> g p n m", p=NF).transpose([1, 0, 2, 3])

    with TileContext(nc) as tc:
        with tc.tile_pool(name="consts", bufs=1) as cpool, \
             tc.tile_pool(name="xs", bufs=len(CHUNKS_T) + 1) as xpool, \
             tc.tile_pool(name="outs", bufs=10) as opool:
            # Ring plan (FIFO order per ring, ~13.3 MB each):
            #   sync  : x0 x2 s(0,0) s(0,2) xe s(1,0) s(1,2) ...
            #   scalar: gb x1 x3 s(0,1) s(0,3) s(1,1) s(1,3) ...
            gbt = cpool.tile([P, G * HW], F16)
            nc.scalar.dma_start(gbt, gb.ap().unsqueeze(0).broadcast_to([P, G * HW]))

            xtiles = []
            chunk_off = []
            n0 = 0
            for i, ns in enumerate(CHUNKS_T):
                xtiles.append(xpool.tile([P, NSUB_MAX * HW], F16, tag="x",
                                         name=f"xt{i}"))
                chunk_off.append(n0)
                n0 += ns

            def loadc(i):
                ns = CHUNKS_T[i]
                eng = nc.sync if i % 2 == 0 else nc.scalar
                eng.dma_start(
                    xtiles[i][:, 0:ns * HW].rearrange("p (n m) -> p n m", m=HW),
                    xc[:, chunk_off[i]:chunk_off[i] + ns, :],
                )

            loadc(0)
            loadc(1)
            loadc(2)
            loadc(3)

            xte = xpool.tile([P, E * HW], F16, tag="x", name="xte")
            xtev = xte.rearrange("p (n m) -> p n m", m=HW)

            def gb_bc(g, ns, pn=P):  # [.., 81] -> [.., ns, 81] step-0 view
                return gbt[0:pn, g * HW:(g + 1) * HW].unsqueeze(1).broadcast_to(
                    [pn, ns, HW]
                )

            # ---- streaming broadcast-multiply, stores alternate rings ----
            s = 0

            def ring():
                nonlocal s
                eng = nc.sync if s % 2 == 0 else nc.scalar
                s += 1
                return eng

            for i, ns in enumerate(CHUNKS_T):
                n0 = chunk_off[i]
                xtv = xtiles[i][:, 0:ns * HW].rearrange("p (n m) -> p n m", m=HW)
                for g in range(G):
                    ot = opool.tile([P, NSUB_MAX * HW], F16, tag="o")
                    otv = ot[:, 0:ns * HW].rearrange("p (n m) -> p n m", m=HW)
                    nc.vector.tensor_tensor(otv, xtv, gb_bc(g, ns), ALU.mult)
                    ring().dma_start(oc[:, g, n0:n0 + ns, :], otv)
                if i == 0:
                    # extra-chunk load enters the sync FIFO here
                    nc.sync.dma_start(xtev[0:NF], xe)

            for g in range(G):
                ot = opool.tile([P, NSUB_MAX * HW], F16, tag="o")
                otv = ot[:, 0:E * HW].rearrange("p (n m) -> p n m", m=HW)
                nc.vector.tensor_tensor(otv[0:NF], xtev[0:NF], gb_bc(g, E, NF),
                                        ALU.mult)
                ring().dma_start(oe[:, g], otv[0:NF])
    nc.finalize()
    return nc


def make_gabor(theta, lam):
    """[G, 81] f32 Gabor filters, mirroring the reference synthesis."""
    ys = np.arange(H, dtype=np.float32) - (H - 1) / 2.0
    xs = np.arange(W, dtype=np.float32) - (W - 1) / 2.0
    y, x = np.meshgrid(ys, xs, indexing="ij")
    th = theta[:, None, None].astype(np.float32)
    l = lam[:, None, None].astype(np.float32)
    xr = x[None] * np.cos(th) + y[None] * np.sin(th)
    yr = -x[None] * np.sin(th) + y[None] * np.cos(th)
    env = np.exp(-(xr ** 2 + yr ** 2) / (2.0 * np.float32(SIGMA) ** 2))
    g = env * np.cos(2.0 * np.float32(np.pi) * xr * l)
    return g.reshape(G, HW).astype(np.float32)


def _dev_order():
    """Original-row index for each device-layout row (length ROWS)."""
    rp = np.full(P, T + E, dtype=np.int64)
    rp[NF:] = T
    a = np.concatenate([[0], np.cumsum(rp)[:-1]])   # first original row per p
    common = (a[:, None] + np.arange(T)[None, :]).reshape(-1)        # [128*T]
    extra = (a[:NF, None] + T + np.arange(E)[None, :]).reshape(-1)   # [120*E]
    return np.concatenate([common, extra])


DEV_ORDER = _dev_order()

_NC = None
TRACE = False          # set True by the local test harness for NTFF timing
LAST_RESULT = None     # BassKernelResults of the most recent run


def kernel(x, theta, lam):
    global _NC
    if _NC is None:
        _NC = build_bass()
    x = np.ascontiguousarray(np.asarray(x, dtype=np.float32))
    theta = np.asarray(theta, dtype=np.float32).reshape(G)
    lam = np.asarray(lam, dtype=np.float32).reshape(G)
    x16 = x.astype(np.float16)
    gb16 = make_gabor(theta, lam).astype(np.float16).reshape(G * HW)

    in_maps = []
    for m in range(N_CORES):
        shard = x16[m * CO_SH:(m + 1) * CO_SH].reshape(ROWS, HW)
        in_maps.append({"x": np.ascontiguousarray(shard[DEV_ORDER]), "gb": gb16})

    global LAST_RESULT
    LAST_RESULT = run_bass_kernel_spmd(
        _NC, in_maps, list(range(N_CORES)), trace=TRACE
    )
    res = LAST_RESULT.results

    out = np.empty((G, CO, CI, H, W), dtype=np.float32)
    shard_out = np.empty((G, ROWS, HW), dtype=np.float32)
    for m in range(N_CORES):
        shard_out[:, DEV_ORDER, :] = res[m]["out"]
        out[:, m * CO_SH:(m + 1) * CO_SH] = shard_out.reshape(G, CO_SH, CI, H, W)
    return out.reshape(G * CO, CI, H, W)
